# revision 27
# baseline (speedup 1.0000x reference)
"""Trainium2 Bass kernel for nn_ExpertsChooseMaskedExpand (MoE routing).

Reference computes (per batch b):
    xd[e,c,j] = sum_t mask[t,e,c] * x[t,e,j]          (dispatch)
    y[e,c,o]  = sum_j xd[e,c,j] * w[e,o,j] + bias[o]  (expert GEMM)
    out[t,o]  = sum_{e,c} comb[t,e,c] * y[e,c,o]      (combine)

We use associativity to contract comb with xd first:
    z[t,e,j] = sum_c comb[t,e,c] * xd[e,c,j]
    out[t,o] = sum_{e,j} z[t,e,j] * w[e,o,j] + bias[o] * S[t],
    S[t] = sum_{e,c} comb[t,e,c]
which cuts FLOPs ~3.4x and never materializes y (B,E,C,O).

Sharding: 8 cores; core k handles batch b=k//2 and expert group
h=k%2 (experts h*4..h*4+4) over ALL 4096 tokens. Each core produces a
partial out (T, O) summed over its 4 experts only; the host adds the
two partials of each batch pair (plus bias*S). This halves both the
dispatch-mask DMA and the dispatch matmul work per core versus
splitting tokens (where dispatch must be duplicated across the pair).

Dispatch runs xh-stationary: one 128x128 ldweights per token tile and
a 512-wide mask stream, producing xd^T[j,c] in PSUM; 16 PE transposes
recover xd[c,j] for the z stage. All matmuls are bf16 with fp32 PSUM
accumulation; partial outputs are stored bf16 (host sums in fp32).

Schedule: the head is DMA-bound on the 16.8MB mask stream (the sync
queue carries mask chunks with the tch0 comb slices riding mid-expert;
x / weight-slices go on the scalar queue, tail comb loads on gpsimd so
they never FIFO behind the mask). The combine tail is PE-bound at the
~216ns/matmul NX issue floor with stage_b interleaved one t-chunk
ahead; psum->sbuf drains are split between the vector and scalar
engines.
"""

import numpy as np
import ml_dtypes

BF16 = ml_dtypes.bfloat16

B, T, E, C = 4, 4096, 8, 512
I = 128            # per-expert input features
O = 4096           # out_features
NCORES = 8
EL = E // 2        # experts per core
NTT = T // 128     # 32 token tiles
NCT = C // 128     # 4 c-tiles
NTCH = T // 512    # 8 t-chunks (z / combine granularity)
NOT2 = O // 1024   # 4 o-slices of the weight DRAM layout

_CACHE = {}


def _build():
    import concourse.bass as bass
    import concourse.tile as tile
    import concourse.bacc as bacc
    import concourse.mybir as mybir

    f32 = mybir.dt.float32
    bf16 = mybir.dt.bfloat16
    ts = bass.ts

    nc = bacc.Bacc(None, target_bir_lowering=False, debug=False)

    xh = nc.dram_tensor("xh", [EL, 128, NTT, I], bf16, kind="ExternalInput")
    mh = nc.dram_tensor("mh", [EL, 128, NTT, C], bf16, kind="ExternalInput")
    cbt = nc.dram_tensor("cbt", [EL, NCT, 128, T], bf16, kind="ExternalInput")
    wf = nc.dram_tensor("wf", [NOT2, 128, EL, 1024], bf16,
                        kind="ExternalInput")
    ident = nc.dram_tensor("ident", [128, 128], bf16, kind="ExternalInput")
    out_d = nc.dram_tensor("out", [T, O], bf16, kind="ExternalOutput")

    with tile.TileContext(nc) as tc:
        with (
            tc.tile_pool(name="persist", bufs=1) as persist,
            tc.tile_pool(name="psumB", bufs=1, space="PSUM") as psumb,
        ):
            wf_sb = persist.tile([128, EL, O], bf16, tag="wf")
            id_sb = persist.tile([128, 128], bf16, tag="ident")
            nc.scalar.dma_start(id_sb[:], ident[:])

            xd = {}   # e -> xd tile [128c, (ct j)] bf16
            zt = {}   # (e, tch) -> z^T tile [128j, 512t] bf16
            pout = {}  # (tt, ot<2) -> staged partial combine over e0..e2

            def cb_load(e, tch, cb_pool, cb_bufs, eng):
                # tail loads ride the gpsimd DMA queue, concurrent with
                # the sync queue's mask stream
                cb_t = cb_pool.tile([128, NCT, 512], bf16, tag="cb",
                                    bufs=cb_bufs, name=f"cb{e}_{tch}")
                for ct in range(NCT):
                    eng.dma_start(cb_t[:, ct, :],
                                  cbt[e, ct, :, ts(tch, 512)])
                return cb_t

            def stage_b(e, tch, cb_t):
                # z^T[e][tch][j, t] = sum_c xd[e][c, j] * comb^T[c, t]
                ps_b = psumb.tile([128, 512], f32, tag="psB", bufs=2,
                                  name=f"psB{e}_{tch}")
                for ct in range(NCT):
                    nc.tensor.matmul(
                        ps_b[:],
                        xd[e][:, ts(ct, 128)],
                        cb_t[:, ct, :],
                        start=(ct == 0),
                        stop=(ct == NCT - 1),
                    )
                z_sb = persist.tile([128, 512], bf16, tag=f"zt{e}_{tch}",
                                    name=f"zt{e}_{tch}")
                nc.vector.tensor_copy(z_sb[:], ps_b[:])
                zt[(e, tch)] = z_sb

            # ---- Head phase: dispatch (DMA-bound on the mask stream) ----
            with (
                tc.tile_pool(name="head", bufs=1) as head,
                tc.tile_pool(name="psumD", bufs=1, space="PSUM") as psumd,
            ):
                for e in range(EL):
                    xh_t = head.tile([128, NTT, I], bf16, tag="xh", bufs=2,
                                     name=f"xh{e}")
                    nc.scalar.dma_start(xh_t[:], xh[e])
                    # xd^T accumulator: [128j, 512c], one chain over all tt
                    ps_d = psumd.tile([128, C], f32, tag="psD", bufs=2,
                                      name=f"psD{e}")
                    for q in range(NTT // 8):
                        mh_t = head.tile([128, 8, C], bf16, tag="mh", bufs=5,
                                         name=f"mh{e}_{q}")
                        nc.sync.dma_start(mh_t[:], mh[e, :, q * 8:q * 8 + 8, :])
                        if q == 2:
                            # cb for tch0 mid-mask: rides ahead of only the
                            # last mask chunk, lands before stage_b needs it
                            cb0_t = cb_load(e, 0, head, 2, nc.sync)
                        for i in range(8):
                            tt = q * 8 + i
                            nc.tensor.matmul(
                                ps_d[:],
                                xh_t[:, tt, :],
                                mh_t[:, i, :],
                                start=(tt == 0),
                                stop=(tt == NTT - 1),
                            )
                    xdT_sb = head.tile([128, C], bf16, tag="xdT", bufs=2,
                                       name=f"xdT{e}")
                    xd_sb = persist.tile([128, C], bf16, tag=f"xd{e}",
                                         name=f"xd{e}")
                    for ct in range(NCT):
                        # scalar engine: idle here, and keeps the cast off
                        # the vector queue so the transpose starts sooner
                        nc.scalar.copy(xdT_sb[:, ts(ct, 128)],
                                       ps_d[:, ts(ct, 128)])
                        ps_t = psumd.tile([128, 128], bf16, tag="psT", bufs=2,
                                          name=f"psT{e}_{ct}")
                        nc.tensor.transpose(ps_t[:], xdT_sb[:, ts(ct, 128)],
                                            id_sb[:])
                        nc.vector.tensor_copy(xd_sb[:, ts(ct, 128)], ps_t[:])
                    xd[e] = xd_sb
                    stage_b(e, 0, cb0_t)
                    # partial combine chains for (tch0, ot0/ot1): experts
                    # e0+e1 accumulate at e1-end (fills e2's mask-wait PE
                    # idle), e2 joins at e2-end (fills e3's); the tail only
                    # injects the staged partial and adds e3. Moves ~7us of
                    # combine work into the DMA-bound head.
                    if e in (1, 2):
                        for tt in range(4):
                            for ot in range(2):
                                ps_p = psumd.tile(
                                    [128, 512], f32, tag="psP", bufs=2,
                                    name=f"psP{e}_{tt}_{ot}")
                                if e == 1:
                                    for pe in (0, 1):
                                        nc.tensor.matmul(
                                            ps_p[:],
                                            zt[(pe, 0)][:, ts(tt, 128)],
                                            wf_sb[:, pe, ts(ot, 512)],
                                            start=(pe == 0), stop=(pe == 1),
                                        )
                                else:
                                    nc.tensor.matmul(
                                        ps_p[:], id_sb[:],
                                        pout[(tt, ot)][:],
                                        start=True, stop=False)
                                    nc.tensor.matmul(
                                        ps_p[:],
                                        zt[(2, 0)][:, ts(tt, 128)],
                                        wf_sb[:, 2, ts(ot, 512)],
                                        start=False, stop=True)
                                po = (head if e == 1 else persist).tile(
                                    [128, 512], bf16, tag=f"po{e}_{tt}_{ot}",
                                    name=f"po{e}_{tt}_{ot}")
                                nc.vector.tensor_copy(po[:], ps_p[:])
                                pout[(tt, ot)] = po
                    # weight o-slices: only the first 1MB slice must be
                    # resident at tail start; the rest queue behind the last
                    # mask bytes on the sync ring and land just ahead of
                    # their first use a few final chains into the tail.
                    if e == 0:
                        nc.scalar.dma_start(wf_sb[:, :, ts(0, 1024)], wf[0])
                    elif e == EL - 1:
                        for ot in (1, 2, 3):
                            nc.sync.dma_start(wf_sb[:, :, ts(ot, 1024)],
                                              wf[ot])

            # ---- Combine phase (PE-bound), stage B interleaved per tch ----
            with (
                tc.tile_pool(name="tail", bufs=1) as tail,
                tc.tile_pool(name="psumC", bufs=1, space="PSUM") as psumc,
            ):
                for tcg in range(NTCH):
                    for tt in range(tcg * 4, tcg * 4 + 4):
                        m = tt % 4
                        out_sb = tail.tile([128, O], bf16, tag="out",
                                           bufs=3, name=f"out{tt}")
                        for ot in range(NOT2 * 2):
                            ps_c = psumc.tile([128, 512], f32, tag="psC",
                                              bufs=5, name=f"psC{tt}_{ot}")
                            if tcg == 0 and ot < 2:
                                nc.tensor.matmul(
                                    ps_c[:], id_sb[:], pout[(tt, ot)][:],
                                    start=True, stop=False)
                                nc.tensor.matmul(
                                    ps_c[:],
                                    zt[(3, 0)][:, ts(m, 128)],
                                    wf_sb[:, 3, ts(ot, 512)],
                                    start=False, stop=True)
                            else:
                                for e in range(EL):
                                    nc.tensor.matmul(
                                        ps_c[:],
                                        zt[(e, tcg)][:, ts(m, 128)],
                                        wf_sb[:, e, ts(ot, 512)],
                                        start=(e == 0),
                                        stop=(e == EL - 1),
                                    )
                            last = tcg == NTCH - 1 and tt == tcg * 4 + 3
                            if ot % 2 == 0 or (last and ot == 7):
                                nc.vector.tensor_copy(
                                    out_sb[:, ts(ot, 512)], ps_c[:])
                            else:
                                nc.scalar.copy(
                                    out_sb[:, ts(ot, 512)], ps_c[:])
                            if last and ot == 5:
                                nc.scalar.dma_start(
                                    out_d[ts(tt, 128), 0:3 * O // 4],
                                    out_sb[:, 0:3 * O // 4])
                        if last:
                            nc.scalar.dma_start(
                                out_d[ts(tt, 128), 3 * O // 4:O],
                                out_sb[:, 3 * O // 4:O])
                        else:
                            nc.scalar.dma_start(out_d[ts(tt, 128), :],
                                                out_sb[:])
                    if tcg + 1 < NTCH:
                        for e in range(EL):
                            stage_b(e, tcg + 1, cb_load(e, tcg + 1, tail, 6, nc.gpsimd))

    nc.compile()
    return nc


def _prep_inputs(x, weight, bias, combine_array, dispatch_mask):
    """Host-side cast to bf16 + re-layout for contiguous device DMA."""
    x = np.asarray(x, np.float32)
    weight = np.asarray(weight, np.float32)
    bias = np.asarray(bias, np.float32)
    comb = np.asarray(combine_array, np.float32)
    mask = np.asarray(dispatch_mask, np.float32)

    # xh: (B, E, 128, NTT, I); xh[b, e, p, tt, j] = x[b, tt*128+p, e, j]
    xh = np.ascontiguousarray(
        x.reshape(B, NTT, 128, E, I).transpose(0, 3, 2, 1, 4)).astype(BF16)
    # mh: (B, E, 128, NTT, C)
    mh = np.ascontiguousarray(
        mask.reshape(B, NTT, 128, E, C).transpose(0, 3, 2, 1, 4)).astype(BF16)
    # cbt: (B, E, NCT, 128, T); [..., e, ct, p, t] = comb[b, t, e, ct*128+p]
    cbt = np.ascontiguousarray(
        comb.reshape(B, T, E, NCT, 128).transpose(0, 2, 3, 4, 1)).astype(BF16)
    # wf: (NOT2, 128, E, 1024); wf[ot, j, e, oq] =
    #     weight.reshape(E, O, I)[e, ot*1024+oq, j]
    wf = np.ascontiguousarray(
        weight.reshape(E, NOT2, 1024, I).transpose(1, 3, 0, 2)).astype(BF16)
    # S[b, t] = sum_{e,c} comb[b, t, e, c] -- bias*S added on host in f32
    s = comb.sum(axis=(2, 3))
    idm = np.eye(128, dtype=BF16)

    in_maps = []
    for k in range(NCORES):
        b, h = k // 2, k % 2
        es = slice(h * EL, (h + 1) * EL)
        in_maps.append({
            "xh": np.ascontiguousarray(xh[b, es]),
            "mh": np.ascontiguousarray(mh[b, es]),
            "cbt": np.ascontiguousarray(cbt[b, es]),
            "wf": np.ascontiguousarray(wf[:, :, es, :]),
            "ident": idm,
        })
    return in_maps, s, bias


def kernel(x, weight, bias, combine_array, dispatch_mask):
    from concourse import bass_utils

    if "nc" not in _CACHE:
        _CACHE["nc"] = _build()
    nc = _CACHE["nc"]

    in_maps, s, bias_f = _prep_inputs(
        x, weight, bias, combine_array, dispatch_mask)
    res = bass_utils.run_bass_kernel_spmd(
        nc, in_maps, core_ids=list(range(NCORES)))
    out = np.empty((B, T, O), np.float32)
    for b in range(B):
        out[b] = res.results[2 * b]["out"].astype(np.float32)
        out[b] += res.results[2 * b + 1]["out"].astype(np.float32)
    out += s[:, :, None] * bias_f[None, None, :]
    return out


# revision 28
# speedup vs baseline: 1.0195x; 1.0195x over previous
"""Trainium2 Bass kernel for nn_ExpertsChooseMaskedExpand (MoE routing).

Reference computes (per batch b):
    xd[e,c,j] = sum_t mask[t,e,c] * x[t,e,j]          (dispatch)
    y[e,c,o]  = sum_j xd[e,c,j] * w[e,o,j] + bias[o]  (expert GEMM)
    out[t,o]  = sum_{e,c} comb[t,e,c] * y[e,c,o]      (combine)

We use associativity to contract comb with xd first:
    z[t,e,j] = sum_c comb[t,e,c] * xd[e,c,j]
    out[t,o] = sum_{e,j} z[t,e,j] * w[e,o,j] + bias[o] * S[t],
    S[t] = sum_{e,c} comb[t,e,c]
which cuts FLOPs ~3.4x and never materializes y (B,E,C,O).

Sharding: 8 cores; core k handles batch b=k//2 and expert group
h=k%2 (experts h*4..h*4+4) over ALL 4096 tokens. Each core produces a
partial out (T, O) summed over its 4 experts only; the host adds the
two partials of each batch pair (plus bias*S). This halves both the
dispatch-mask DMA and the dispatch matmul work per core versus
splitting tokens (where dispatch must be duplicated across the pair).

Dispatch runs xh-stationary: one 128x128 ldweights per token tile and
a 512-wide mask stream, producing xd^T[j,c] in PSUM; 16 PE transposes
recover xd[c,j] for the z stage. All matmuls are bf16 with fp32 PSUM
accumulation; partial outputs are stored bf16 (host sums in fp32).

Schedule: the head is DMA-bound on the 16.8MB mask stream (the sync
queue carries mask chunks with the tch0 comb slices riding mid-expert;
x / weight-slices go on the scalar queue, tail comb loads on gpsimd so
they never FIFO behind the mask). The combine tail is PE-bound at the
~216ns/matmul NX issue floor with stage_b interleaved one t-chunk
ahead; psum->sbuf drains are split between the vector and scalar
engines.
"""

import numpy as np
import ml_dtypes

BF16 = ml_dtypes.bfloat16

B, T, E, C = 4, 4096, 8, 512
I = 128            # per-expert input features
O = 4096           # out_features
NCORES = 8
EL = E // 2        # experts per core
NTT = T // 128     # 32 token tiles
NCT = C // 128     # 4 c-tiles
NTCH = T // 512    # 8 t-chunks (z / combine granularity)
NOT2 = O // 1024   # 4 o-slices of the weight DRAM layout

_CACHE = {}


def _build():
    import concourse.bass as bass
    import concourse.tile as tile
    import concourse.bacc as bacc
    import concourse.mybir as mybir

    f32 = mybir.dt.float32
    bf16 = mybir.dt.bfloat16
    ts = bass.ts

    nc = bacc.Bacc(None, target_bir_lowering=False, debug=False)

    xh = nc.dram_tensor("xh", [EL, 128, NTT, I], bf16, kind="ExternalInput")
    mh = nc.dram_tensor("mh", [EL, 128, NTT, C], bf16, kind="ExternalInput")
    cbt = nc.dram_tensor("cbt", [EL, 128, NCT, T], bf16,
                         kind="ExternalInput")
    wf = nc.dram_tensor("wf", [NOT2, 128, EL, 1024], bf16,
                        kind="ExternalInput")
    ident = nc.dram_tensor("ident", [128, 128], bf16, kind="ExternalInput")
    out_d = nc.dram_tensor("out", [T, O], bf16, kind="ExternalOutput")

    with tile.TileContext(nc) as tc:
        with (
            tc.tile_pool(name="persist", bufs=1) as persist,
            tc.tile_pool(name="psumB", bufs=1, space="PSUM") as psumb,
        ):
            wf_sb = persist.tile([128, EL, O], bf16, tag="wf")
            id_sb = persist.tile([128, 128], bf16, tag="ident")
            nc.scalar.dma_start(id_sb[:], ident[:])

            xd = {}   # e -> xd tile [128c, (ct j)] bf16
            zt = {}   # (e, tch) -> z^T tile [128j, 512t] bf16
            pout = {}  # (tt, ot<2) -> staged partial combine over e0..e2

            def cb_load(e, tch, cb_pool, cb_bufs, eng):
                # tail loads ride the gpsimd DMA queue, concurrent with
                # the sync queue's mask stream
                cb_t = cb_pool.tile([128, NCT, 512], bf16, tag="cb",
                                    bufs=cb_bufs, name=f"cb{e}_{tch}")
                eng.dma_start(cb_t[:], cbt[e, :, :, ts(tch, 512)])
                return cb_t

            def stage_b(e, tch, cb_t):
                # z^T[e][tch][j, t] = sum_c xd[e][c, j] * comb^T[c, t]
                ps_b = psumb.tile([128, 512], f32, tag="psB", bufs=2,
                                  name=f"psB{e}_{tch}")
                for ct in range(NCT):
                    nc.tensor.matmul(
                        ps_b[:],
                        xd[e][:, ts(ct, 128)],
                        cb_t[:, ct, :],
                        start=(ct == 0),
                        stop=(ct == NCT - 1),
                    )
                z_sb = persist.tile([128, 512], bf16, tag=f"zt{e}_{tch}",
                                    name=f"zt{e}_{tch}")
                nc.vector.tensor_copy(z_sb[:], ps_b[:])
                zt[(e, tch)] = z_sb

            # ---- Head phase: dispatch (DMA-bound on the mask stream) ----
            with (
                tc.tile_pool(name="head", bufs=1) as head,
                tc.tile_pool(name="psumD", bufs=1, space="PSUM") as psumd,
            ):
                for e in range(EL):
                    xh_t = head.tile([128, NTT, I], bf16, tag="xh", bufs=2,
                                     name=f"xh{e}")
                    nc.scalar.dma_start(xh_t[:], xh[e])
                    # xd^T accumulator: [128j, 512c], one chain over all tt
                    ps_d = psumd.tile([128, C], f32, tag="psD", bufs=2,
                                      name=f"psD{e}")
                    for q in range(NTT // 8):
                        mh_t = head.tile([128, 8, C], bf16, tag="mh", bufs=5,
                                         name=f"mh{e}_{q}")
                        nc.sync.dma_start(mh_t[:], mh[e, :, q * 8:q * 8 + 8, :])
                        if q == 2:
                            # cb for tch0 mid-mask: rides ahead of only the
                            # last mask chunk, lands before stage_b needs it
                            cb0_t = cb_load(e, 0, head, 2, nc.sync)
                        for i in range(8):
                            tt = q * 8 + i
                            nc.tensor.matmul(
                                ps_d[:],
                                xh_t[:, tt, :],
                                mh_t[:, i, :],
                                start=(tt == 0),
                                stop=(tt == NTT - 1),
                            )
                    xdT_sb = head.tile([128, C], bf16, tag="xdT", bufs=2,
                                       name=f"xdT{e}")
                    xd_sb = persist.tile([128, C], bf16, tag=f"xd{e}",
                                         name=f"xd{e}")
                    for ct in range(NCT):
                        # scalar engine: idle here, and keeps the cast off
                        # the vector queue so the transpose starts sooner
                        nc.scalar.copy(xdT_sb[:, ts(ct, 128)],
                                       ps_d[:, ts(ct, 128)])
                        ps_t = psumd.tile([128, 128], bf16, tag="psT", bufs=2,
                                          name=f"psT{e}_{ct}")
                        nc.tensor.transpose(ps_t[:], xdT_sb[:, ts(ct, 128)],
                                            id_sb[:])
                        nc.vector.tensor_copy(xd_sb[:, ts(ct, 128)], ps_t[:])
                    xd[e] = xd_sb
                    stage_b(e, 0, cb0_t)
                    # partial combine chains for (tch0, ot0/ot1): experts
                    # e0+e1 accumulate at e1-end (fills e2's mask-wait PE
                    # idle), e2 joins at e2-end (fills e3's); the tail only
                    # injects the staged partial and adds e3. Moves ~7us of
                    # combine work into the DMA-bound head.
                    if e in (1, 2):
                        for tt in range(4):
                            for ot in range(2):
                                ps_p = psumd.tile(
                                    [128, 512], f32, tag="psP", bufs=2,
                                    name=f"psP{e}_{tt}_{ot}")
                                if e == 1:
                                    for pe in (0, 1):
                                        nc.tensor.matmul(
                                            ps_p[:],
                                            zt[(pe, 0)][:, ts(tt, 128)],
                                            wf_sb[:, pe, ts(ot, 512)],
                                            start=(pe == 0), stop=(pe == 1),
                                        )
                                else:
                                    nc.tensor.matmul(
                                        ps_p[:], id_sb[:],
                                        pout[(tt, ot)][:],
                                        start=True, stop=False)
                                    nc.tensor.matmul(
                                        ps_p[:],
                                        zt[(2, 0)][:, ts(tt, 128)],
                                        wf_sb[:, 2, ts(ot, 512)],
                                        start=False, stop=True)
                                po = (head if e == 1 else persist).tile(
                                    [128, 512], bf16, tag=f"po{e}_{tt}_{ot}",
                                    name=f"po{e}_{tt}_{ot}")
                                nc.vector.tensor_copy(po[:], ps_p[:])
                                pout[(tt, ot)] = po
                    # weight o-slices: only the first 1MB slice must be
                    # resident at tail start; the rest queue behind the last
                    # mask bytes on the sync ring and land just ahead of
                    # their first use a few final chains into the tail.
                    if e == 0:
                        nc.scalar.dma_start(wf_sb[:, :, ts(0, 1024)], wf[0])
                    elif e == EL - 1:
                        for ot in (1, 2, 3):
                            nc.sync.dma_start(wf_sb[:, :, ts(ot, 1024)],
                                              wf[ot])

            # ---- Combine phase (PE-bound), stage B interleaved per tch ----
            with (
                tc.tile_pool(name="tail", bufs=1) as tail,
                tc.tile_pool(name="psumC", bufs=1, space="PSUM") as psumc,
            ):
                for tcg in range(NTCH):
                    for tt in range(tcg * 4, tcg * 4 + 4):
                        m = tt % 4
                        out_sb = tail.tile([128, O], bf16, tag="out",
                                           bufs=3, name=f"out{tt}")
                        for ot in range(NOT2 * 2):
                            ps_c = psumc.tile([128, 512], f32, tag="psC",
                                              bufs=5, name=f"psC{tt}_{ot}")
                            if tcg == 0 and ot < 2:
                                nc.tensor.matmul(
                                    ps_c[:], id_sb[:], pout[(tt, ot)][:],
                                    start=True, stop=False)
                                nc.tensor.matmul(
                                    ps_c[:],
                                    zt[(3, 0)][:, ts(m, 128)],
                                    wf_sb[:, 3, ts(ot, 512)],
                                    start=False, stop=True)
                            else:
                                for e in range(EL):
                                    nc.tensor.matmul(
                                        ps_c[:],
                                        zt[(e, tcg)][:, ts(m, 128)],
                                        wf_sb[:, e, ts(ot, 512)],
                                        start=(e == 0),
                                        stop=(e == EL - 1),
                                    )
                            last = tcg == NTCH - 1 and tt == tcg * 4 + 3
                            if ot % 2 == 0 or (last and ot == 7):
                                nc.vector.tensor_copy(
                                    out_sb[:, ts(ot, 512)], ps_c[:])
                            else:
                                nc.scalar.copy(
                                    out_sb[:, ts(ot, 512)], ps_c[:])
                            if last and ot == 5:
                                nc.scalar.dma_start(
                                    out_d[ts(tt, 128), 0:3 * O // 4],
                                    out_sb[:, 0:3 * O // 4])
                        if last:
                            nc.scalar.dma_start(
                                out_d[ts(tt, 128), 3 * O // 4:O],
                                out_sb[:, 3 * O // 4:O])
                        else:
                            nc.scalar.dma_start(out_d[ts(tt, 128), :],
                                                out_sb[:])
                        if tcg + 1 < NTCH:
                            # one z chain per token tile keeps the psB /
                            # vector load smooth instead of bunching all
                            # four at the group boundary
                            e = tt % 4
                            stage_b(e, tcg + 1,
                                    cb_load(e, tcg + 1, tail, 6, nc.gpsimd))

    nc.compile()
    return nc


def _prep_inputs(x, weight, bias, combine_array, dispatch_mask):
    """Host-side cast to bf16 + re-layout for contiguous device DMA."""
    x = np.asarray(x, np.float32)
    weight = np.asarray(weight, np.float32)
    bias = np.asarray(bias, np.float32)
    comb = np.asarray(combine_array, np.float32)
    mask = np.asarray(dispatch_mask, np.float32)

    # xh: (B, E, 128, NTT, I); xh[b, e, p, tt, j] = x[b, tt*128+p, e, j]
    xh = np.ascontiguousarray(
        x.reshape(B, NTT, 128, E, I).transpose(0, 3, 2, 1, 4)).astype(BF16)
    # mh: (B, E, 128, NTT, C)
    mh = np.ascontiguousarray(
        mask.reshape(B, NTT, 128, E, C).transpose(0, 3, 2, 1, 4)).astype(BF16)
    # cbt: (B, E, 128, NCT, T); [..., e, p, ct, t] = comb[b, t, e, ct*128+p]
    cbt = np.ascontiguousarray(
        comb.reshape(B, T, E, NCT, 128).transpose(0, 2, 4, 3, 1)).astype(BF16)
    # wf: (NOT2, 128, E, 1024); wf[ot, j, e, oq] =
    #     weight.reshape(E, O, I)[e, ot*1024+oq, j]
    wf = np.ascontiguousarray(
        weight.reshape(E, NOT2, 1024, I).transpose(1, 3, 0, 2)).astype(BF16)
    # S[b, t] = sum_{e,c} comb[b, t, e, c] -- bias*S added on host in f32
    s = comb.sum(axis=(2, 3))
    idm = np.eye(128, dtype=BF16)

    in_maps = []
    for k in range(NCORES):
        b, h = k // 2, k % 2
        es = slice(h * EL, (h + 1) * EL)
        in_maps.append({
            "xh": np.ascontiguousarray(xh[b, es]),
            "mh": np.ascontiguousarray(mh[b, es]),
            "cbt": np.ascontiguousarray(cbt[b, es]),
            "wf": np.ascontiguousarray(wf[:, :, es, :]),
            "ident": idm,
        })
    return in_maps, s, bias


def kernel(x, weight, bias, combine_array, dispatch_mask):
    from concourse import bass_utils

    if "nc" not in _CACHE:
        _CACHE["nc"] = _build()
    nc = _CACHE["nc"]

    in_maps, s, bias_f = _prep_inputs(
        x, weight, bias, combine_array, dispatch_mask)
    res = bass_utils.run_bass_kernel_spmd(
        nc, in_maps, core_ids=list(range(NCORES)))
    out = np.empty((B, T, O), np.float32)
    for b in range(B):
        out[b] = res.results[2 * b]["out"].astype(np.float32)
        out[b] += res.results[2 * b + 1]["out"].astype(np.float32)
    out += s[:, :, None] * bias_f[None, None, :]
    return out


# revision 31
# speedup vs baseline: 1.0467x; 1.0267x over previous
"""Trainium2 Bass kernel for nn_ExpertsChooseMaskedExpand (MoE routing).

Reference computes (per batch b):
    xd[e,c,j] = sum_t mask[t,e,c] * x[t,e,j]          (dispatch)
    y[e,c,o]  = sum_j xd[e,c,j] * w[e,o,j] + bias[o]  (expert GEMM)
    out[t,o]  = sum_{e,c} comb[t,e,c] * y[e,c,o]      (combine)

We use associativity to contract comb with xd first:
    z[t,e,j] = sum_c comb[t,e,c] * xd[e,c,j]
    out[t,o] = sum_{e,j} z[t,e,j] * w[e,o,j] + bias[o] * S[t],
    S[t] = sum_{e,c} comb[t,e,c]
which cuts FLOPs ~3.4x and never materializes y (B,E,C,O).

Sharding: 8 cores; core k handles batch b=k//2 and expert group
h=k%2 (experts h*4..h*4+4) over ALL 4096 tokens. Each core produces a
partial out (T, O) summed over its 4 experts only; the host adds the
two partials of each batch pair (plus bias*S). This halves both the
dispatch-mask DMA and the dispatch matmul work per core versus
splitting tokens (where dispatch must be duplicated across the pair).

Dispatch runs xh-stationary: one 128x128 ldweights per token tile and
a 512-wide mask stream, producing xd^T[j,c] in PSUM; 16 PE transposes
recover xd[c,j] for the z stage. All matmuls are bf16 with fp32 PSUM
accumulation; partial outputs are stored bf16 (host sums in fp32).

Schedule: the head is DMA-bound on the 16.8MB mask stream (the sync
queue carries mask chunks with the tch0 comb slices riding mid-expert;
x / weight-slices go on the scalar queue, tail comb loads on gpsimd so
they never FIFO behind the mask). The combine tail is PE-bound at the
~216ns/matmul NX issue floor with stage_b interleaved one t-chunk
ahead; psum->sbuf drains are split between the vector and scalar
engines.
"""

import numpy as np
import ml_dtypes

BF16 = ml_dtypes.bfloat16

B, T, E, C = 4, 4096, 8, 512
I = 128            # per-expert input features
O = 4096           # out_features
NCORES = 8
EL = E // 2        # experts per core
NTT = T // 128     # 32 token tiles
NCT = C // 128     # 4 c-tiles
NTCH = T // 512    # 8 t-chunks (z / combine granularity)
NOT2 = O // 1024   # 4 o-slices of the weight DRAM layout

_CACHE = {}


def _build():
    import concourse.bass as bass
    import concourse.tile as tile
    import concourse.bacc as bacc
    import concourse.mybir as mybir

    f32 = mybir.dt.float32
    bf16 = mybir.dt.bfloat16
    ts = bass.ts

    nc = bacc.Bacc(None, target_bir_lowering=False, debug=False)

    xh = nc.dram_tensor("xh", [EL, 128, NTT, I], bf16, kind="ExternalInput")
    mh = nc.dram_tensor("mh", [EL, 128, NTT, C], bf16, kind="ExternalInput")
    cbt = nc.dram_tensor("cbt", [EL, 128, NCT, T], bf16,
                         kind="ExternalInput")
    # head-critical tch0 comb slice, host-packed contiguous for 4KB bursts
    cb0_d = nc.dram_tensor("cb0", [EL, 128, NCT * 512], bf16,
                           kind="ExternalInput")
    wf = nc.dram_tensor("wf", [NOT2, 128, EL, 1024], bf16,
                        kind="ExternalInput")
    ident = nc.dram_tensor("ident", [128, 128], bf16, kind="ExternalInput")
    out_d = nc.dram_tensor("out", [T, O], bf16, kind="ExternalOutput")

    with tile.TileContext(nc) as tc:
        with (
            tc.tile_pool(name="persist", bufs=1) as persist,
            tc.tile_pool(name="psumB", bufs=1, space="PSUM") as psumb,
        ):
            wf_sb = persist.tile([128, EL, O], bf16, tag="wf")
            id_sb = persist.tile([128, 128], bf16, tag="ident")
            nc.scalar.dma_start(id_sb[:], ident[:])

            xd = {}   # e -> xd tile [128c, (ct j)] bf16
            zt = {}   # (e, tch) -> z^T tile [128j, 512t] bf16
            pout = {}  # (tt, ot<2) -> staged partial combine over e0..e2

            def cb_load(e, tch, cb_pool, cb_bufs, eng):
                # tail loads ride the gpsimd DMA queue, concurrent with
                # the sync queue's mask stream
                cb_t = cb_pool.tile([128, NCT, 512], bf16, tag="cb",
                                    bufs=cb_bufs, name=f"cb{e}_{tch}")
                eng.dma_start(cb_t[:], cbt[e, :, :, ts(tch, 512)])
                return cb_t

            def stage_b(e, tch, cb_t):
                # z^T[e][tch][j, t] = sum_c xd[e][c, j] * comb^T[c, t]
                ps_b = psumb.tile([128, 512], f32, tag="psB", bufs=2,
                                  name=f"psB{e}_{tch}")
                for ct in range(NCT):
                    nc.tensor.matmul(
                        ps_b[:],
                        xd[e][:, ts(ct, 128)],
                        cb_t[:, ct, :],
                        start=(ct == 0),
                        stop=(ct == NCT - 1),
                    )
                z_sb = persist.tile([128, 512], bf16, tag=f"zt{e}_{tch}",
                                    name=f"zt{e}_{tch}")
                nc.vector.tensor_copy(z_sb[:], ps_b[:])
                zt[(e, tch)] = z_sb

            # ---- Head phase: dispatch (DMA-bound on the mask stream) ----
            with (
                tc.tile_pool(name="head", bufs=1) as head,
                tc.tile_pool(name="psumD", bufs=1, space="PSUM") as psumd,
            ):
                for e in range(EL):
                    xh_t = head.tile([128, NTT, I], bf16, tag="xh", bufs=2,
                                     name=f"xh{e}")
                    nc.scalar.dma_start(xh_t[:], xh[e])
                    # xd^T accumulator: [128j, 512c], one chain over all tt
                    ps_d = psumd.tile([128, C], f32, tag="psD", bufs=2,
                                      name=f"psD{e}")
                    for q in range(NTT // 8):
                        mh_t = head.tile([128, 8, C], bf16, tag="mh", bufs=10,
                                         name=f"mh{e}_{q}")
                        nc.sync.dma_start(mh_t[:], mh[e, :, q * 8:q * 8 + 8, :])
                        if q == 1:
                            # cb for tch0 mid-mask: rides ahead of only the
                            # last mask chunk, lands before stage_b needs it
                            cb0_t = head.tile([128, NCT, 512], bf16,
                                              tag="cb", bufs=2,
                                              name=f"cb{e}_0")
                            nc.sync.dma_start(cb0_t[:], cb0_d[e])
                        for i in range(8):
                            tt = q * 8 + i
                            nc.tensor.matmul(
                                ps_d[:],
                                xh_t[:, tt, :],
                                mh_t[:, i, :],
                                start=(tt == 0),
                                stop=(tt == NTT - 1),
                            )
                    xdT_sb = head.tile([128, C], bf16, tag="xdT", bufs=2,
                                       name=f"xdT{e}")
                    xd_sb = persist.tile([128, C], bf16, tag=f"xd{e}",
                                         name=f"xd{e}")
                    for ct in range(NCT):
                        # scalar engine: idle here, and keeps the cast off
                        # the vector queue so the transpose starts sooner
                        nc.scalar.copy(xdT_sb[:, ts(ct, 128)],
                                       ps_d[:, ts(ct, 128)])
                        ps_t = psumd.tile([128, 128], bf16, tag="psT", bufs=2,
                                          name=f"psT{e}_{ct}")
                        nc.tensor.transpose(ps_t[:], xdT_sb[:, ts(ct, 128)],
                                            id_sb[:])
                        nc.vector.tensor_copy(xd_sb[:, ts(ct, 128)], ps_t[:])
                    xd[e] = xd_sb
                    stage_b(e, 0, cb0_t)
                    # partial combine chains for (tch0, ot0/ot1): experts
                    # e0+e1 accumulate at e1-end (fills e2's mask-wait PE
                    # idle), e2 joins at e2-end (fills e3's); the tail only
                    # injects the staged partial and adds e3. Moves ~7us of
                    # combine work into the DMA-bound head.
                    if e in (1, 2):
                        for tt in range(4):
                            for ot in range(2):
                                ps_p = psumd.tile(
                                    [128, 512], f32, tag="psP", bufs=2,
                                    name=f"psP{e}_{tt}_{ot}")
                                if e == 1:
                                    for pe in (0, 1):
                                        nc.tensor.matmul(
                                            ps_p[:],
                                            zt[(pe, 0)][:, ts(tt, 128)],
                                            wf_sb[:, pe, ts(ot, 512)],
                                            start=(pe == 0), stop=(pe == 1),
                                        )
                                else:
                                    nc.tensor.matmul(
                                        ps_p[:], id_sb[:],
                                        pout[(tt, ot)][:],
                                        start=True, stop=False)
                                    nc.tensor.matmul(
                                        ps_p[:],
                                        zt[(2, 0)][:, ts(tt, 128)],
                                        wf_sb[:, 2, ts(ot, 512)],
                                        start=False, stop=True)
                                po = (head if e == 1 else persist).tile(
                                    [128, 512], bf16, tag=f"po{e}_{tt}_{ot}",
                                    name=f"po{e}_{tt}_{ot}")
                                nc.vector.tensor_copy(po[:], ps_p[:])
                                pout[(tt, ot)] = po
                    # weight o-slices: only the first 1MB slice must be
                    # resident at tail start; the rest queue behind the last
                    # mask bytes on the sync ring and land just ahead of
                    # their first use a few final chains into the tail.
                    if e == 0:
                        nc.scalar.dma_start(wf_sb[:, :, ts(0, 1024)], wf[0])
                    elif e == EL - 1:
                        for ot in (1, 2, 3):
                            nc.sync.dma_start(wf_sb[:, :, ts(ot, 1024)],
                                              wf[ot])

            # ---- Combine phase (PE-bound), stage B interleaved per tch ----
            with (
                tc.tile_pool(name="tail", bufs=1) as tail,
                tc.tile_pool(name="psumC", bufs=1, space="PSUM") as psumc,
            ):
                for tcg in range(NTCH):
                    for tt in range(tcg * 4, tcg * 4 + 4):
                        m = tt % 4
                        out_sb = tail.tile([128, O], bf16, tag="out",
                                           bufs=3, name=f"out{tt}")
                        for ot in range(NOT2 * 2):
                            ps_c = psumc.tile([128, 512], f32, tag="psC",
                                              bufs=5, name=f"psC{tt}_{ot}")
                            if tcg == 0 and ot < 2:
                                nc.tensor.matmul(
                                    ps_c[:], id_sb[:], pout[(tt, ot)][:],
                                    start=True, stop=False)
                                nc.tensor.matmul(
                                    ps_c[:],
                                    zt[(3, 0)][:, ts(m, 128)],
                                    wf_sb[:, 3, ts(ot, 512)],
                                    start=False, stop=True)
                            else:
                                for e in range(EL):
                                    nc.tensor.matmul(
                                        ps_c[:],
                                        zt[(e, tcg)][:, ts(m, 128)],
                                        wf_sb[:, e, ts(ot, 512)],
                                        start=(e == 0),
                                        stop=(e == EL - 1),
                                    )
                            last = tcg == NTCH - 1 and tt == tcg * 4 + 3
                            if ot % 2 == 0 or (last and ot == 7):
                                nc.vector.tensor_copy(
                                    out_sb[:, ts(ot, 512)], ps_c[:])
                            else:
                                nc.scalar.copy(
                                    out_sb[:, ts(ot, 512)], ps_c[:])
                            if last and ot == 5:
                                nc.scalar.dma_start(
                                    out_d[ts(tt, 128), 0:3 * O // 4],
                                    out_sb[:, 0:3 * O // 4])
                        if last:
                            nc.scalar.dma_start(
                                out_d[ts(tt, 128), 3 * O // 4:O],
                                out_sb[:, 3 * O // 4:O])
                        else:
                            nc.scalar.dma_start(out_d[ts(tt, 128), :],
                                                out_sb[:])
                        if tcg + 1 < NTCH:
                            # one z chain per token tile keeps the psB /
                            # vector load smooth instead of bunching all
                            # four at the group boundary
                            e = tt % 4
                            stage_b(e, tcg + 1,
                                    cb_load(e, tcg + 1, tail, 6, nc.gpsimd))

    nc.compile()
    return nc


def _prep_inputs(x, weight, bias, combine_array, dispatch_mask):
    """Host-side cast to bf16 + re-layout for contiguous device DMA."""
    x = np.asarray(x, np.float32)
    weight = np.asarray(weight, np.float32)
    bias = np.asarray(bias, np.float32)
    comb = np.asarray(combine_array, np.float32)
    mask = np.asarray(dispatch_mask, np.float32)

    # xh: (B, E, 128, NTT, I); xh[b, e, p, tt, j] = x[b, tt*128+p, e, j]
    xh = np.ascontiguousarray(
        x.reshape(B, NTT, 128, E, I).transpose(0, 3, 2, 1, 4)).astype(BF16)
    # mh: (B, E, 128, NTT, C)
    mh = np.ascontiguousarray(
        mask.reshape(B, NTT, 128, E, C).transpose(0, 3, 2, 1, 4)).astype(BF16)
    # cbt: (B, E, 128, NCT, T); [..., e, p, ct, t] = comb[b, t, e, ct*128+p]
    cbt = np.ascontiguousarray(
        comb.reshape(B, T, E, NCT, 128).transpose(0, 2, 4, 3, 1)).astype(BF16)
    # cb0: (B, E, 128, NCT*512) -- packed copy of cbt[..., 0:512]
    cb0 = np.ascontiguousarray(cbt[:, :, :, :, 0:512]).reshape(
        B, E, 128, NCT * 512)
    # wf: (NOT2, 128, E, 1024); wf[ot, j, e, oq] =
    #     weight.reshape(E, O, I)[e, ot*1024+oq, j]
    wf = np.ascontiguousarray(
        weight.reshape(E, NOT2, 1024, I).transpose(1, 3, 0, 2)).astype(BF16)
    # S[b, t] = sum_{e,c} comb[b, t, e, c] -- bias*S added on host in f32
    s = comb.sum(axis=(2, 3))
    idm = np.eye(128, dtype=BF16)

    in_maps = []
    for k in range(NCORES):
        b, h = k // 2, k % 2
        es = slice(h * EL, (h + 1) * EL)
        in_maps.append({
            "xh": np.ascontiguousarray(xh[b, es]),
            "mh": np.ascontiguousarray(mh[b, es]),
            "cbt": np.ascontiguousarray(cbt[b, es]),
            "cb0": np.ascontiguousarray(cb0[b, es]),
            "wf": np.ascontiguousarray(wf[:, :, es, :]),
            "ident": idm,
        })
    return in_maps, s, bias


def kernel(x, weight, bias, combine_array, dispatch_mask):
    from concourse import bass_utils

    if "nc" not in _CACHE:
        _CACHE["nc"] = _build()
    nc = _CACHE["nc"]

    in_maps, s, bias_f = _prep_inputs(
        x, weight, bias, combine_array, dispatch_mask)
    res = bass_utils.run_bass_kernel_spmd(
        nc, in_maps, core_ids=list(range(NCORES)))
    out = np.empty((B, T, O), np.float32)
    for b in range(B):
        out[b] = res.results[2 * b]["out"].astype(np.float32)
        out[b] += res.results[2 * b + 1]["out"].astype(np.float32)
    out += s[:, :, None] * bias_f[None, None, :]
    return out
